# revision 1
# baseline (speedup 1.0000x reference)
# MoE (GShard top-1, capacity=S/E) inference kernel for Trainium2, 8 cores.
# Expert-parallel MLP + data-parallel gate with AllGather'd logits and
# fully on-device routing (cumsum positions via triangular matmuls,
# dispatch/combine via indirect DMA gather/scatter).
import sys

sys.path.insert(0, "/opt/trn_rl_repo")

import numpy as np
import ml_dtypes

import concourse.bass as bass
import concourse.mybir as mybir
import concourse.bacc as bacc
import concourse.tile as tile
from concourse.bass_utils import run_bass_kernel_spmd

S, M, H, E = 8192, 1024, 4096, 8
C = S // E  # 1024 capacity
NCORES = 8
TPC = S // NCORES  # tokens per core shard = 1024
NA = S // 128  # 64 token tiles of 128
NA_LOC = TPC // 128  # 8 local token tiles
F32 = mybir.dt.float32
BF16 = mybir.dt.bfloat16
I16 = mybir.dt.int16
BF = ml_dtypes.bfloat16

X = mybir.AxisListType.X
OP = mybir.AluOpType
ACTF = mybir.ActivationFunctionType


def _build_program():
    nc = bacc.Bacc(
        "TRN2",
        target_bir_lowering=False,
        debug=False,
        num_devices=NCORES,
        dynamic_dma_scratch_size=32768,
        num_swdge_queues=2,
    )

    # ---- I/O ----
    din = {}
    for name, shape, dt in [
        ("xsh", [TPC, M], F32),          # this core's token shard (fp32, gate)
        ("xg", [S + 1, M], BF16),        # full x (bf16) + zero row 0, for gather
        ("wg", [M, E], F32),
        ("w1", [M, H], BF16),            # this core's expert inter_w
        ("b1", [H], F32),
        ("w2", [H, M], BF16),            # this core's expert output_w
        ("identf", [128, 128], F32),
        ("identb", [128, 128], BF16),
        ("tri", [128, 128], F32),        # tri[k,p] = 1 if k < p
        ("ones_k", [128, 1], F32),       # column of ones (partition reduce)
        ("ones_p", [1, 128], F32),       # row of ones (partition broadcast)
        ("iota_e", [128, NA * E], F32),  # tile(0..7) per token slot
        ("trash", [128, NA], F32),       # 1024 + (t % 1024)
        ("tokc", [128, NA], F32),        # t + 1
        ("cid", [128, 1], F32),          # this core's index
        ("b2bc", [128, M], F32),         # b2 replicated across partitions
    ]:
        din[name] = nc.dram_tensor(name, shape, dt, kind="ExternalInput")

    out_e = nc.dram_tensor("outE", [C, M], F32, kind="ExternalOutput")
    out_meta = nc.dram_tensor("outmeta", [C, 2], F32, kind="ExternalOutput")

    with tile.TileContext(nc) as tc:
        _kernel_body(nc, tc, din, out_e, out_meta)

    nc.compile()
    return nc


def _kernel_body(nc, tc, din, out_e, out_meta):
    from contextlib import ExitStack

    stack = ExitStack()
    cpool = stack.enter_context(tc.tile_pool(name="const", bufs=1))
    dram = stack.enter_context(tc.tile_pool(name="dram", bufs=1, space="DRAM"))

    def cload(name, shape, dt=F32, src=None):
        t = cpool.tile(shape, dt, tag=name, name=name)
        nc.sync.dma_start(t[:], src if src is not None else din[name].ap())
        return t

    identf = cload("identf", [128, 128])
    wg_sb = cload("wg", [128, M // 128, E], src=din["wg"].ap().rearrange("(kb p) e -> p kb e", p=128))

    # DRAM scratch. Logits stored as the SBUF image [p=t%128, a, e] so the
    # shard->global transport is pure contiguous runs.
    # Two slot buffers so the two scatter halves don't serialize.
    loglocs = [dram.tile([128, NA_LOC // 2, E], F32, name=f"logloc{h}") for h in range(2)]
    logfulls = [
        dram.tile([NCORES, 128, NA_LOC // 2, E], F32, name=f"logfull{h}")
        for h in range(2)
    ]
    slotbufs = [dram.tile([2 * C, 64], F32, name=f"slotbuf{h}") for h in range(2)]

    # ================= Phase G: gate logits (fp32) =================
    # logits[t, e] = sum_m x[t, m] * wg[m, e]; needs xT tiles on PE.
    with tc.tile_pool(name="gate", bufs=1) as gpool, \
         tc.tile_pool(name="gpsum", bufs=5, space="PSUM") as gpsum, \
         tc.tile_pool(name="lpsum", bufs=2, space="PSUM") as lpsum:
        # per token tile: transpose its 8 m-blocks, then its 8 gate matmuls —
        # tile a+1's transposes overlap tile a's matmuls
        lloc = cpool.tile([128, NA_LOC, E], F32, tag="lloc", name="lloc")
        for a in range(NA_LOC):
            xs = gpool.tile([128, M], F32, tag="xs", name="xs", bufs=4)
            nc.sync.dma_start(xs[:], din["xsh"].ap()[a * 128:(a + 1) * 128, :])
            xt = gpool.tile([128, M // 128, 128], F32, tag="xt", name="xt", bufs=2)
            for kg in range(2):
                pt = gpsum.tile([128, 512], F32, tag="gtp", name="gtp")
                for k4 in range(4):
                    kb = kg * 4 + k4
                    nc.tensor.transpose(
                        pt[:, k4 * 128:(k4 + 1) * 128],
                        xs[:, kb * 128:(kb + 1) * 128],
                        identf[:],
                    )
                nc.vector.tensor_copy(
                    xt[:, kg * 4:(kg + 1) * 4, :].rearrange("p a b -> p (a b)"),
                    pt[:],
                )
            lps = lpsum.tile([128, E], F32, tag="lps", name="lps")
            for kb in range(M // 128):
                nc.tensor.matmul(
                    lps[:], xt[:, kb, :], wg_sb[:, kb, :],
                    start=(kb == 0), stop=(kb == M // 128 - 1),
                )
            nc.vector.tensor_copy(lloc[:, a, :], lps[:])

    identb = cload("identb", [128, 128], BF16)
    tri = cload("tri", [128, 128])
    ones_k = cload("ones_k", [128, 1])
    ones_p = cload("ones_p", [1, 128])
    iota_e = cload("iota_e", [128, NA * E])
    trash = cload("trash", [128, NA])
    tokc = cload("tokc", [128, NA])
    cid = cload("cid", [128, 1])
    b1_sb = cload("b1", [128, H // 128], src=din["b1"].ap().rearrange("(hb p) -> p hb", p=128))
    b2bc = cload("b2bc", [128, M])

    # two half AllGathers: the first overlaps the gate's second half
    for h in range(2):
        nc.sync.dma_start(
            loglocs[h][:], lloc[:, h * (NA_LOC // 2):(h + 1) * (NA_LOC // 2), :]
        )
        nc.gpsimd.collective_compute(
            "AllGather",
            OP.bypass,
            replica_groups=[list(range(NCORES))],
            ins=[loglocs[h][:]],
            outs=[logfulls[h][:]],
        )

    # ================= Phase R: routing (all tokens, redundant) =====
    rstack = ExitStack()
    rpool = rstack.enter_context(tc.tile_pool(name="rt", bufs=1))
    rpsum = rstack.enter_context(tc.tile_pool(name="rpsum", bufs=1, space="PSUM"))

    def rt(tag, shape=(128, NA * E), dt=F32):
        return rpool.tile(list(shape), dt, tag=tag, name=tag)

    # Per-AG-half routing prefix: separate L tiles per half so half-0's
    # softmax/argmax chain overlaps AG2's transfer. Half h covers local tiles
    # h*4..h*4+3 of every rank, i.e. global tiles g with (g mod 8) in that
    # range — strided slices of the (d, a, e)-ordered full tensors.
    Lh = []
    for h in range(2):
        L_h = rt(f"L{h}", (128, NA * E // 2))
        for dd in range(NCORES):
            (nc.sync if dd % 2 == 0 else nc.scalar).dma_start(
                L_h[:, dd * 32:(dd + 1) * 32],
                logfulls[h][dd].rearrange("p a e -> p (a e)"),
            )
        Lh.append(L_h)

    mx = rt("mx", (128, NA))
    lm = rt("lm")
    ex = rt("ex")
    se = rt("se", (128, NA))
    gatev = rt("gatev", (128, NA))
    mask1 = rt("mask1")
    mask13 = mask1[:].rearrange("p (a e) -> p a e", e=E)
    for h in range(2):
        ha = slice(h * 4, (h + 1) * 4)
        L4 = Lh[h][:].rearrange("p (d a e) -> p d a e", d=NCORES, e=E)
        mxh = mx[:].rearrange("p (d a) -> p d a", d=NCORES)[:, :, ha]
        nc.vector.reduce_max(mxh, L4, axis=X)
        mxbh = mxh.unsqueeze(3).broadcast_to([128, NCORES, 4, E])
        lmh = lm[:].rearrange("p (d a e) -> p d a e", d=NCORES, e=E)[:, :, ha, :]
        nc.vector.tensor_tensor(lmh, L4, mxbh, op=OP.subtract)
        exh = ex[:].rearrange("p (d a e) -> p d a e", d=NCORES, e=E)[:, :, ha, :]
        nc.scalar.activation(exh, lmh, ACTF.Exp)
        seh = se[:].rearrange("p (d a) -> p d a", d=NCORES)[:, :, ha]
        nc.vector.reduce_sum(seh.unsqueeze(3), exh, axis=X)
        nc.vector.reciprocal(
            gatev[:].rearrange("p (d a) -> p d a", d=NCORES)[:, :, ha], seh
        )
        # argmax mask. Exact fp32 ties are absent for this input distribution
        # (verified: output error matches pure-bf16 MLP noise).
        m1h = mask1[:].rearrange("p (d a e) -> p d a e", d=NCORES, e=E)[:, :, ha, :]
        nc.vector.tensor_tensor(m1h, L4, mxbh, op=OP.is_equal)

    # exclusive cumsum over all tokens: per-tile tri matmul + tile offsets
    totp = rpsum.tile([1, NA * E], F32, tag="totp", name="totp")
    nc.tensor.matmul(totp[:], ones_k[:], mask1[:], start=True, stop=True)
    tot = rt("tot", (1, NA * E))
    nc.vector.tensor_copy(tot[:], totp[:])

    # exclusive scan of per-tile totals across the 64 tiles (stride E)
    cur = tot
    for k in (1, 2, 4, 8, 16, 32):
        nxt = rt(f"sc{k}", (1, NA * E))
        nc.vector.tensor_copy(nxt[:], cur[:])
        c3 = cur[:].rearrange("p (a e) -> p a e", e=E)
        n3 = nxt[:].rearrange("p (a e) -> p a e", e=E)
        nc.vector.tensor_tensor(n3[:, k:NA, :], c3[:, k:NA, :], c3[:, 0:NA - k, :], op=OP.add)
        cur = nxt
    exc = rt("exc", (1, NA * E))
    nc.vector.tensor_tensor(exc[:], cur[:], tot[:], op=OP.subtract)

    # in-tile exclusive cumsum for ALL 64 tiles at once (tri matmul is linear
    # over the free dim), then the per-tile offsets as one rank-1 update.
    locp = rpsum.tile([128, NA * E], F32, tag="locp", name="locp")
    nc.tensor.matmul(locp[:], tri[:], mask1[:], start=True, stop=False)
    nc.tensor.matmul(locp[:], ones_p[:], exc[:], start=False, stop=True)

    loc = rt("loc")
    nc.vector.tensor_copy(loc[:], locp[:])

    # m1k = mask1 * (loc < C) in one fused op
    m1k = rt("m1k")
    m1k3 = m1k[:].rearrange("p (a e) -> p a e", e=E)
    nc.vector.scalar_tensor_tensor(
        m1k[:], loc[:], float(C), mask1[:], op0=OP.is_lt, op1=OP.mult
    )

    posm = rt("posm")
    nc.vector.tensor_tensor(posm[:], loc[:], m1k[:], op=OP.mult)
    pos = rt("pos", (128, NA))
    nc.vector.reduce_sum(pos[:].unsqueeze(2), posm[:].rearrange("p (a e) -> p a e", e=E), axis=X)
    kept = rt("kept", (128, NA))
    nc.vector.reduce_sum(kept[:].unsqueeze(2), m1k3, axis=X)
    eidm = rt("eidm")
    nc.vector.tensor_tensor(eidm[:], iota_e[:], m1k[:], op=OP.mult)
    eid = rt("eid", (128, NA))
    nc.vector.reduce_sum(eid[:].unsqueeze(2), eidm[:].rearrange("p (a e) -> p a e", e=E), axis=X)

    ism = rt("ism", (128, NA))
    nc.vector.tensor_scalar(ism[:], eid[:], cid[:, 0:1], None, op0=OP.is_equal)
    vm = rt("vm", (128, NA))
    nc.vector.tensor_tensor(vm[:], ism[:], kept[:], op=OP.mult)

    # off = trash + vm * (pos - trash)   (select without CopyPredicated)
    offd = rt("offd", (128, NA))
    nc.vector.tensor_tensor(offd[:], pos[:], trash[:], op=OP.subtract)
    offm = rt("offm", (128, NA))
    nc.vector.tensor_tensor(offm[:], offd[:], vm[:], op=OP.mult)
    off = rt("off", (128, NA))
    nc.vector.tensor_tensor(off[:], offm[:], trash[:], op=OP.add)

    # Wrap-16 shuffle on-chip: token t = a*128 + g*16 + q must land at
    # idx8[q, a*8+g]. Double PE transpose with a free-dim permute between:
    #   off [p=(g,q), a] -T-> [a, (g,q)] -permute-> [a, (q,g)] -T-> [(q,g), a]
    # then partition-strided SBUF->SBUF DMAs assemble [16, (a, g)].
    engs = [nc.sync, nc.scalar]
    ofT = rt("ofT", (64, 128), I16)
    ofp = rpsum.tile([64, 128], F32, tag="ofp", name="ofp")
    nc.tensor.transpose(ofp[:], off[:], identf[:])
    nc.vector.tensor_copy(
        ofT[:].rearrange("a (q g) -> a q g", g=8),
        ofp[:].rearrange("a (g q) -> a q g", q=16),
    )
    idx8 = rt("idx8", (128, S // 16), I16)
    for q in range(16):
        engs[q % 2].dma_start(idx8[q:q + 1, :], ofT[:, q * 8:(q + 1) * 8])
    for g in range(1, 8):
        engs[g % 2].dma_start(idx8[16 * g:16 * (g + 1), :], idx8[0:16, :])

    # payload (token_id+1, gate) per token
    pay = rt("pay", (128, NA, 64))
    nc.vector.memset(pay[:, :, :], 0.0)
    nc.vector.tensor_copy(pay[:, :, 0:1], tokc[:].unsqueeze(2))
    nc.vector.tensor_copy(pay[:, :, 1:2], gatev[:].unsqueeze(2))

    # zero the slot buffers, then scatter each token half into its own buffer
    # (disjoint outputs let the two scatter DMAs overlap)
    zt = rt("zt", (128, 512))
    nc.vector.memset(zt[:], 0.0)
    for h in range(2):
        nc.sync.dma_start(
            slotbufs[h][0:C, :].rearrange("(a b) c -> a (b c)", a=128), zt[:]
        )
    for h in range(2):
        nh = NA // 2
        nc.gpsimd.dma_scatter_add(
            slotbufs[h][:, 0:64],
            pay[:, h * nh:(h + 1) * nh, :],
            idx8[:, h * (S // 32):(h + 1) * (S // 32)],
            S // 2,
            S // 2,
            64,
            queue_num=h,
        )

    # read back per-slot (token, gate): each half-buffer holds the slots its
    # tokens landed in (zeros elsewhere) — sum of the two is the full table
    srd = []
    for h in range(2):
        s_h = rt(f"srd{h}", (128, C // 128, 2))
        engs[h].dma_start(
            s_h[:, :, :],
            slotbufs[h][0:C, 0:2].rearrange("(j p) two -> p j two", p=128),
        )
        srd.append(s_h)
    slotrd = cpool.tile([128, C // 128, 2], F32, tag="slotrd", name="slotrd")
    nc.vector.tensor_tensor(slotrd[:, :, :], srd[0][:, :, :], srd[1][:, :, :], op=OP.add)
    gate_slot = cpool.tile([128, C // 128, 1], F32, tag="gate_slot", name="gate_slot")
    nc.vector.tensor_copy(gate_slot[:, :, :], slotrd[:, :, 1:2])
    # wrap-16 the token column for the gather idx list (same double transpose:
    # tok[p=(g,q), j] -> [(q,g), j] -> strided assemble [16, (j, g)])
    tkT = rt("tkT", (8, 128), I16)
    tkp = rpsum.tile([8, 128], F32, tag="tkp", name="tkp")
    nc.tensor.transpose(tkp[:], slotrd[:, :, 0], identf[:])
    nc.vector.tensor_copy(
        tkT[:].rearrange("j (q g) -> j q g", g=8),
        tkp[:].rearrange("j (g q) -> j q g", q=16),
    )
    tok16 = cpool.tile([128, C // 16], I16, tag="tok16", name="tok16")
    for q in range(16):
        engs[q % 2].dma_start(tok16[q:q + 1, :], tkT[:, q * 8:(q + 1) * 8])
    for g in range(1, 8):
        engs[g % 2].dma_start(tok16[16 * g:16 * (g + 1), :], tok16[0:16, :])
    nc.sync.dma_start(
        out_meta.ap()[:].rearrange("(j p) two -> p j two", p=128),
        slotrd[:, :, :],
    )

    # gather this expert's token rows pre-transposed (16-bit transpose mode):
    # gx[p, kb, c] = x[tok_c, kb*128 + p] — directly usable as MM1's rhs.
    # gather token rows in two halves so MM1 can start on the first half
    gxrs = []
    for h in range(2):
        gxr_h = cpool.tile([128, C // 256, M], BF16, tag=f"gxr{h}", name=f"gxr{h}")
        nc.gpsimd.dma_gather(
            gxr_h[:, :, :],
            din["xg"].ap(),
            tok16[:, h * (C // 32):(h + 1) * (C // 32)],
            C // 2,
            C // 2,
            M,
            queue_num=h,
        )
        gxrs.append(gxr_h)

    rstack.close()

    # ================= Phase M: expert MLP =================
    NJ = C // 128  # 8 c-blocks
    NKB = M // 128  # 8 m-blocks
    NHB = H // 128  # 32 h-blocks

    mpool = stack.enter_context(tc.tile_pool(name="mlp", bufs=1))
    wpool = stack.enter_context(tc.tile_pool(name="wstream", bufs=6))
    opool = stack.enter_context(tc.tile_pool(name="out", bufs=3))

    # dispxT per half: gxh[h][p=m%128, kb, c-in-half]
    gxh = [
        cpool.tile([128, M // 128, C // 2], BF16, tag=f"gxh{h}", name=f"gxh{h}")
        for h in range(2)
    ]
    with tc.tile_pool(name="tpsum", bufs=4, space="PSUM") as tpsum:
        for h in range(2):
            for kb in range(NKB):
                ptt = tpsum.tile([128, 512], BF16, tag="dtp", name="dtp")
                for j4 in range(4):
                    nc.tensor.transpose(
                        ptt[:, j4 * 128:(j4 + 1) * 128],
                        gxrs[h][:, j4, kb * 128:(kb + 1) * 128],
                        identb[:],
                    )
                nc.vector.tensor_copy(gxh[h][:, kb, :], ptt[:])

    mpsum = stack.enter_context(tc.tile_pool(name="mpsum", bufs=8, space="PSUM"))

    # MM1: hT[hb][h, c] = gelu(w1.T @ dispxT + b1)
    hts = [mpool.tile([128, C], BF16, tag=f"ht{hb}", name=f"ht{hb}") for hb in range(NHB)]
    for hp in range(NHB // 2):  # 16 rounds of 2 h-blocks
        pss = [[mpsum.tile([128, 512], F32, tag="mmp", name="mmp") for _ in range(2)] for _ in range(2)]
        for kb in range(NKB):
            w1t = wpool.tile([128, 256], BF16, tag="w1t", name="w1t")
            nc.sync.dma_start(
                w1t[:],
                din["w1"].ap()[kb * 128:(kb + 1) * 128, hp * 256:(hp + 1) * 256],
            )
            for h2 in range(2):
                for ch in range(2):
                    nc.tensor.matmul(
                        pss[h2][ch][:],
                        w1t[:, h2 * 128:(h2 + 1) * 128],
                        gxh[ch][:, kb, :],
                        start=(kb == 0),
                        stop=(kb == NKB - 1),
                    )
        for h2 in range(2):
            hb = hp * 2 + h2
            for ch in range(2):
                nc.scalar.activation(
                    hts[hb][:, ch * 512:(ch + 1) * 512],
                    pss[h2][ch][:],
                    ACTF.Gelu,
                    bias=b1_sb[:, hb:hb + 1],
                )

    # preload w2 (after MM1 trace position so SBUF peaks stay low)
    w2s = [mpool.tile([128, M], BF16, tag=f"w2{hb}", name=f"w2{hb}") for hb in range(NHB)]
    for hb in range(NHB):
        nc.sync.dma_start(w2s[hb][:], din["w2"].ap()[hb * 128:(hb + 1) * 128, :])

    # MM2: out[c, m] = (hT.T @ w2 + b2) * gate
    for jc in range(NJ):
        ops_ = [mpsum.tile([128, 512], F32, tag="mmp", name="mmp") for _ in range(2)]
        for hb in range(NHB):
            for mh in range(2):
                nc.tensor.matmul(
                    ops_[mh][:],
                    hts[hb][:, jc * 128:(jc + 1) * 128],
                    w2s[hb][:, mh * 512:(mh + 1) * 512],
                    start=(hb == 0),
                    stop=(hb == NHB - 1),
                )
        osb = opool.tile([128, M], F32, tag="osb", name="osb")
        for mh in range(2):
            sl = slice(mh * 512, (mh + 1) * 512)
            # out = (psum + b2) * gate
            nc.vector.tensor_tensor(osb[:, sl], ops_[mh][:], b2bc[:, sl], op=OP.add)
            nc.vector.tensor_scalar(
                osb[:, sl], osb[:, sl], gate_slot[:, jc:jc + 1, 0], None, op0=OP.mult
            )
        nc.sync.dma_start(out_e.ap()[jc * 128:(jc + 1) * 128, :], osb[:])

    stack.close()


_NC_CACHE = {}


def _get_nc():
    if "nc" not in _NC_CACHE:
        _NC_CACHE["nc"] = _build_program()
    return _NC_CACHE["nc"]


def _host_consts():
    t = (np.arange(NA)[None, :] * 128 + np.arange(128)[:, None]).astype(np.int64)
    return {
        "identf": np.eye(128, dtype=np.float32),
        "identb": np.eye(128).astype(BF),
        "tri": (np.arange(128)[:, None] < np.arange(128)[None, :]).astype(np.float32),
        "ones_k": np.ones((128, 1), np.float32),
        "ones_p": np.ones((1, 128), np.float32),
        "iota_e": np.tile(np.arange(E, dtype=np.float32), (128, NA)),
        "trash": (C + (t % C)).astype(np.float32),
        "tokc": (t + 1).astype(np.float32),
    }


def kernel(x, wg, inter_w, inter_b, output_w, output_b):
    x = np.asarray(x, np.float32)
    wg = np.asarray(wg, np.float32)
    inter_w = np.asarray(inter_w, np.float32)
    inter_b = np.asarray(inter_b, np.float32)
    output_w = np.asarray(output_w, np.float32)
    output_b = np.asarray(output_b, np.float32)

    nc = _get_nc()
    consts = _host_consts()
    xg = np.concatenate([np.zeros((1, M), np.float32), x]).astype(BF)

    in_maps = []
    for d in range(NCORES):
        in_maps.append(
            {
                "xsh": x[d * TPC:(d + 1) * TPC],
                "xg": xg,
                "wg": wg,
                "w1": inter_w[d].astype(BF),
                "b1": inter_b[d],
                "w2": output_w[d].astype(BF),
                "b2bc": np.tile(output_b[d], (128, 1)),
                "cid": np.full((128, 1), d, np.float32),
                **consts,
            }
        )

    res = run_bass_kernel_spmd(nc, in_maps, list(range(NCORES)))

    y = np.zeros((S, M), np.float32)
    for d in range(NCORES):
        meta = res.results[d]["outmeta"]
        oute = res.results[d]["outE"]
        tok1 = np.rint(meta[:, 0]).astype(np.int64)
        valid = tok1 > 0
        y[tok1[valid] - 1] = oute[valid]
    return y


if __name__ == "__main__":
    pass

